# revision 1
# baseline (speedup 1.0000x reference)
"""Clifford ISTA kernel for 8 Trainium2 NeuronCores.

Strategy (data-parallel, zero cross-core communication):
  - Shard batch B=64 across 8 cores (8 per core).
  - Never materialize the 32 MB Cayley-fused operators. Instead exploit
    K_fwd = A (x) CayleyTable structure: per output blade k,
        Ax_k = sum_j s(k^j, j) * X_{k^j} @ A_j^T
        G_k  = sum_j rev[j] * s(k^j, j) * Err_{k^j} @ A_j
    The signed XOR-permutation over blades is folded into a constant
    signed-permutation matrix Pi [64, 512]: a small PE matmul
    x_chunk.T @ Pi produces all 8 signed/permuted stationary copies at
    once (fused transpose + blade permute + sign), then the main matmuls
    stream SBUF-resident A-derived constants as the moving operand,
    accumulating the blade reduction directly in PSUM (fp32).
  - Matmul operands in bf16 (full-rate PE, fast weight loads); fp32 PSUM
    accumulate, fp32 x-state and fp32 update arithmetic (bf16 state or
    bf16-staged y would cost ~1e-2 accuracy; measured).
  - Soft threshold as u - clamp(u, -thr, +thr) in one DVE tensor_scalar;
    fp32 state write offloaded to GPSIMD; PSUM->SBUF copies split DVE/ACT.
  - 50 iterations fully unrolled; iteration 0 specializes Ax=0 -> err=-y.
  - Measured on HW (interleaved wall-clock deltas): ~10-11 us/iteration,
    ~0.5-0.55 ms per 50-iteration solve; rel err vs reference 1.8e-3.
"""

import os
import numpy as np
import ml_dtypes

# Problem constants (hardcoded per contest contract).
B, M, N, NB = 64, 256, 512, 8
BL = 8           # local batch per core
NCORES = 8
N_ITER = 50
STEP = 0.01
LAMBDAS = [0.0, 0.001, 0.001, 0.002]

# Two PE column-groups => two concurrent moving streams. Measured slower on
# HW than a single stream (weight loads can't pull ahead across groups), so
# default off.
COL_TILE = os.environ.get("COL_TILE", "0") == "1"
# Matmul operand dtype: "bf16" or "f32r".
MM_DT = os.environ.get("MM_DT", "bf16")
# Row-pack the K=64 PREP matmuls into two 64-row groups of the PE array.
# Costs extra DVE duplicate-writes; PE row-group concurrency unverified on
# this HW (col-group packing measured slower), so default off.
ROW_PACK = os.environ.get("ROW_PACK", "0") == "1"


def _cayley_sign():
    """cay[a, b] = C[a, b, a^b] for Cl(3,0): the canonical reordering sign."""
    cay = np.zeros((NB, NB), np.float32)
    for a in range(NB):
        for b in range(NB):
            cnt, aa = 0, a >> 1
            while aa:
                cnt += bin(aa & b).count("1")
                aa >>= 1
            cay[a, b] = -1.0 if (cnt & 1) else 1.0
    return cay


def _grades():
    return np.array([bin(i).count("1") for i in range(NB)], np.int32)


def build_consts(A):
    """Host-side constant tensors shared by all cores (np.float32)."""
    A = np.asarray(A, np.float32)
    cay = _cayley_sign()
    rev = ((-1.0) ** (_grades() * (_grades() - 1) // 2)).astype(np.float32)

    # Pi [64, 512]: Pi[i*8+b', j*64+k*8+b] = cay[i, j] iff i == k^j and b' == b
    pi = np.zeros((NB * BL, NB * NB * BL), np.float32)
    for j in range(NB):
        for k in range(NB):
            i = k ^ j
            s = cay[i, j]
            for b in range(BL):
                pi[i * BL + b, (j * NB + k) * BL + b] = s

    # atf [128, 8192]: atf[p, j*1024 + q*256 + m] = A[m, 128q+p, j]
    At = A.transpose(1, 2, 0).reshape(4, 128, NB, M)       # [q, p, j, m]
    atf = np.ascontiguousarray(At.transpose(1, 2, 0, 3)).reshape(128, 8192)

    # abw [128, 8192]: abw[p, j*1024 + r*512 + n] = A[128r+p, n, j]*rev[j]*STEP
    Ab = A.reshape(2, 128, N, NB)                          # [r, p, n, j]
    abw = np.ascontiguousarray(
        Ab.transpose(1, 3, 0, 2) * (rev * STEP)[None, :, None, None]
    ).reshape(128, 8192)

    # thr [64, 1]: per-blade threshold on partitions (i, b)
    thr_blades = np.array(LAMBDAS, np.float32)[_grades()]  # [8]
    pthr = np.repeat(thr_blades, BL)[:, None].astype(np.float32)

    return pi, atf, abw, pthr


def build_program(n_iter=N_ITER, col_tile=None, mm_dt=None, reps=1,
                  row_pack=None):
    """Build the per-core Bass/Tile program (identical on all cores).

    reps > 1 wraps the whole n_iter body in a hardware loop — timing-only
    builds (the repeated passes keep iterating the converged state).
    """
    from contextlib import ExitStack
    import concourse.bass as bass
    import concourse.tile as tile
    from concourse import bacc, mybir

    if col_tile is None:
        col_tile = COL_TILE
    if mm_dt is None:
        mm_dt = MM_DT
    if row_pack is None:
        row_pack = ROW_PACK
    NH = 2 if col_tile else 1  # number of PE column-groups

    f32 = mybir.dt.float32
    dtm = mybir.dt.bfloat16 if mm_dt == "bf16" else mybir.dt.float32r
    assert not (col_tile and mm_dt != "bf16"), "col-tiling needs bf16"
    ALU = mybir.AluOpType

    nc = bacc.Bacc(None, target_bir_lowering=False)

    pi_d = nc.dram_tensor("pi", [128, 512], dtm, kind="ExternalInput")
    atf_d = nc.dram_tensor("atf", [128, 8192], dtm, kind="ExternalInput")
    abw_d = nc.dram_tensor("abw", [128, 8192], dtm, kind="ExternalInput")
    nyt_d = nc.dram_tensor("nyt", [64, 256], f32, kind="ExternalInput")
    pthr_d = nc.dram_tensor("pthr", [64, 1], f32, kind="ExternalInput")
    nthr_d = nc.dram_tensor("nthr", [64, 1], f32, kind="ExternalInput")
    xout_d = nc.dram_tensor("xout", [64, 512], f32, kind="ExternalOutput")

    with ExitStack() as ctx:
        tc = ctx.enter_context(tile.TileContext(nc))
        cpool = ctx.enter_context(tc.tile_pool(name="consts", bufs=1))
        wpool = ctx.enter_context(tc.tile_pool(name="work", bufs=2))
        ppool = ctx.enter_context(tc.tile_pool(name="ps", bufs=1, space="PSUM"))

        # ---- constant loads (split for DMA-queue parallelism) ----
        pi_t = cpool.tile([128, 512], dtm, name="pi_t")
        nc.sync.dma_start(pi_t[:], pi_d[:])
        nyt_t = cpool.tile([64, 256], f32, name="nyt_t")
        nc.sync.dma_start(nyt_t[:], nyt_d[:])
        pthr_t = cpool.tile([64, 1], f32, name="pthr_t")
        nc.sync.dma_start(pthr_t[:], pthr_d[:])
        nthr_t = cpool.tile([64, 1], f32, name="nthr_t")
        nc.sync.dma_start(nthr_t[:], nthr_d[:])
        abw_t = cpool.tile([128, 8192], dtm, name="abw_t")
        for ch in range(8):
            sl = slice(1024 * ch, 1024 * (ch + 1))
            nc.sync.dma_start(abw_t[:, sl], abw_d[:, sl])
        atf_t = cpool.tile([128, 8192], dtm, name="atf_t")
        for ch in range(8):
            sl = slice(1024 * ch, 1024 * (ch + 1))
            nc.sync.dma_start(atf_t[:, sl], atf_d[:, sl])

        XP = 128 if row_pack else 64   # x_bf/err rows (duplicated if packed)
        x_kb = cpool.tile([64, 512], f32, name="x_kb")     # fp32 state
        nc.vector.memset(x_kb[:], 0.0)
        x_bf = cpool.tile([XP, 512], dtm, name="x_bf")     # matmul shadow
        nc.vector.memset(x_bf[:], 0.0)
        err0_t = cpool.tile([XP, 256], dtm, name="err0_t")
        nc.vector.tensor_copy(err0_t[0:64, :], nyt_t[:])   # bf16 cast of -y
        if row_pack:
            nc.vector.tensor_copy(err0_t[64:128, :], nyt_t[:])

        def copy_halves(dst, src, both_act=False):
            """PSUM->SBUF copy split across DVE and ACT halves. both_act
            puts both halves on ACT to relieve DVE (the busier engine)."""
            if both_act:
                nc.scalar.copy(dst[:, 0:256], src[:, 0:256])
            else:
                nc.vector.tensor_copy(dst[:, 0:256], src[:, 0:256])
            nc.scalar.copy(dst[:, 256:512], src[:, 256:512])

        def psum_pair(base, free, tag, bufs, it):
            """Per-column-group accumulators: separate tiles => separate
            PSUM banks, so Tile never serializes the two groups."""
            if NH == 2:
                top = ppool.tile([64, free], f32, name=f"{base}t_{it}",
                                 tag=tag, bufs=bufs)
                botc = ppool.tile([128, free], f32, name=f"{base}b_{it}",
                                  tag=tag, bufs=bufs)
                return [top[:, :], botc[64:128, :]]
            t = ppool.tile([64, free], f32, name=f"{base}t_{it}",
                           tag=tag, bufs=bufs)
            return [t[:, :]]

        psS_BUFS = 2 if col_tile else 3
        AX_TAG, AX_BUFS = ("pmix", 2) if col_tile else ("psmix", 3)
        PT_TAG, PT_BUFS = ("pmix", 2) if col_tile else ("psT", 2)
        PG_TAG, PG_BUFS = ("psg", 4) if col_tile else ("psmix", 3)

        def emit_iteration(it):
            if it == 0:
                err_ap = err0_t  # x=0 -> Ax=0 -> err = -y
            else:
                # ---- PREP-F: psS[q] = x_chunk_q.T @ Pi; when row-packed,
                # q pairs run on PE row-groups 0-63 / 64-127 concurrently ----
                psS = []
                for q in range(4):
                    ps = ppool.tile([128, 512], f32, name=f"psS{q}_{it}",
                                    tag="psS", bufs=psS_BUFS)
                    rp = slice(64, 128) if (row_pack and q % 2) else slice(0, 64)
                    nc.tensor.matmul(ps[:],
                                     lhsT=x_bf[rp, 128 * q:128 * (q + 1)],
                                     rhs=pi_t[rp, :], start=True, stop=True)
                    psS.append(ps)
                S = []
                for q in range(4):
                    s_t = wpool.tile([128, 512], dtm, name=f"S{q}_{it}",
                                     tag=f"S{q}", bufs=3)
                    copy_halves(s_t, psS[q], both_act=(q in (1, 3)))
                    S.append(s_t)
                # ---- FWD mains: accumulate psAx over (j, q); NH col-groups ----
                axp = psum_pair("psAx", 256, AX_TAG, AX_BUFS, it)
                pairs = [(j, q) for q in range(4) for j in range(8)]
                npair = len(pairs)
                for idx, (j, q) in enumerate(pairs):
                    h = idx % NH
                    nc.tensor.matmul(
                        axp[h],
                        lhsT=S[q][:, 64 * j:64 * (j + 1)],
                        rhs=atf_t[:, 1024 * j + 256 * q:1024 * j + 256 * (q + 1)],
                        start=(idx < NH), stop=(idx >= npair - NH),
                    )
                # ---- ERR: err = sum_h psAx[h] + (-y), chunked by r ----
                err_t = wpool.tile([XP, 256], dtm, name=f"err_{it}",
                                   tag="err", bufs=3)
                for r in range(2):
                    sl = slice(128 * r, 128 * (r + 1))
                    if NH == 2:
                        etmp = wpool.tile([64, 128], f32, name=f"etmp{r}_{it}",
                                          tag=f"etmp{r}", bufs=2)
                        nc.vector.tensor_add(etmp[:], axp[1][:, sl],
                                             nyt_t[:, sl])
                        nc.vector.tensor_add(err_t[0:64, sl], axp[0][:, sl],
                                             etmp[:])
                    else:
                        nc.vector.tensor_add(err_t[0:64, sl], axp[0][:, sl],
                                             nyt_t[:, sl])
                    if row_pack:
                        # duplicate rows for the 64-127 row-group PREP-B
                        nc.vector.tensor_add(
                            err_t[64:128, sl], axp[0][:, sl],
                            etmp[:] if NH == 2 else nyt_t[:, sl])
                err_ap = err_t

            # ---- PREP-B: psT[r] = err_chunk_r.T @ Pi ----
            psT = []
            for r in range(2):
                ps = ppool.tile([128, 512], f32, name=f"psT{r}_{it}",
                                tag=PT_TAG, bufs=PT_BUFS)
                rp = slice(64, 128) if (row_pack and r % 2) else slice(0, 64)
                nc.tensor.matmul(ps[:], lhsT=err_ap[rp, 128 * r:128 * (r + 1)],
                                 rhs=pi_t[rp, :], start=True, stop=True)
                psT.append(ps)
            T = []
            for r in range(2):
                t_t = wpool.tile([128, 512], dtm, name=f"T{r}_{it}",
                                 tag=f"T{r}", bufs=3)
                copy_halves(t_t, psT[r])
                T.append(t_t)
            # ---- BWD mains: psG[nch] = STEP*grad n-chunk; 2 banks so the
            # update of chunk 0 overlaps the bwd matmuls of chunk 1 ----
            psG = []
            for nch in range(2):
                pgp = psum_pair(f"psG{nch}", 256, PG_TAG, PG_BUFS, it)
                pairs_b = [(j, r) for r in range(2) for j in range(8)]
                npb = len(pairs_b)
                for idx, (j, r) in enumerate(pairs_b):
                    h = idx % NH
                    base = 1024 * j + 512 * r + 256 * nch
                    nc.tensor.matmul(
                        pgp[h],
                        lhsT=T[r][:, 64 * j:64 * (j + 1)],
                        rhs=abw_t[:, base:base + 256],
                        start=(idx < NH), stop=(idx >= npb - NH),
                    )
                psG.append(pgp)
            # ---- UPDATE: x = u - clamp(u, -thr, thr), u = x - sum_h psG ----
            for cp in range(4):
                sl = slice(128 * cp, 128 * (cp + 1))
                gsl = slice(128 * (cp % 2), 128 * (cp % 2) + 128)
                u = wpool.tile([64, 128], f32, name=f"u_{cp}_{it}",
                               tag="u", bufs=4)
                if NH == 2:
                    t1 = wpool.tile([64, 128], f32, name=f"t1_{cp}_{it}",
                                    tag="t1", bufs=2)
                    nc.vector.tensor_sub(t1[:], x_kb[:, sl], psG[cp // 2][0][:, gsl])
                    nc.vector.tensor_sub(u[:], t1[:], psG[cp // 2][1][:, gsl])
                else:
                    nc.vector.tensor_sub(u[:], x_kb[:, sl], psG[cp // 2][0][:, gsl])
                c = wpool.tile([64, 128], f32, name=f"c_{cp}_{it}",
                               tag="c", bufs=4)
                nc.vector.tensor_scalar(c[:], u[:], nthr_t[:], pthr_t[:],
                                        ALU.max, ALU.min)
                nc.vector.tensor_sub(x_bf[0:64, sl], u[:], c[:])
                if row_pack:
                    nc.vector.tensor_sub(x_bf[64:128, sl], u[:], c[:])
                # fp32 state write is off the critical path (read only by
                # next iteration's u) and SBUF-only -> idle GPSIMD
                nc.gpsimd.tensor_sub(x_kb[:, sl], u[:], c[:])

        if reps > 1:
            with tc.For_i(0, reps, 1):
                for it in range(n_iter):
                    emit_iteration(it)
        else:
            for it in range(n_iter):
                emit_iteration(it)

        nc.sync.dma_start(xout_d[:], x_kb[:])

    nc.compile()
    return nc


_program_cache = {}


def _get_program(n_iter):
    if n_iter not in _program_cache:
        _program_cache[n_iter] = build_program(n_iter)
    return _program_cache[n_iter]


LAST_INFO = {}


def kernel(y, A, _trace=False, _n_iter=None):
    y = np.asarray(y, np.float32)
    A = np.asarray(A, np.float32)
    n_iter = N_ITER if _n_iter is None else _n_iter

    from concourse.bass_utils import run_bass_kernel_spmd

    nc = _get_program(n_iter)
    pi, atf, abw, pthr = build_consts(A)
    pi2 = np.concatenate([pi, pi], axis=0)                 # both row-groups
    mdt = ml_dtypes.bfloat16 if MM_DT == "bf16" else np.float32
    pi_m, atf_m, abw_m = pi2.astype(mdt), atf.astype(mdt), abw.astype(mdt)

    in_maps = []
    for c in range(NCORES):
        ysl = y[BL * c:BL * (c + 1)]                       # [8, 256, 8] (b, m, k)
        nyt = np.ascontiguousarray(-ysl.transpose(2, 0, 1).reshape(NB * BL, M))
        in_maps.append({
            "pi": pi_m, "atf": atf_m, "abw": abw_m, "nyt": nyt,
            "pthr": pthr, "nthr": -pthr,
        })

    try:
        res = run_bass_kernel_spmd(
            nc, in_maps, core_ids=list(range(NCORES)), trace=_trace,
        )
    except ModuleNotFoundError:
        # NTFF profile hook unavailable in this container; run untraced.
        res = run_bass_kernel_spmd(
            nc, in_maps, core_ids=list(range(NCORES)), trace=False,
        )
    LAST_INFO["exec_time_ns"] = res.exec_time_ns
    LAST_INFO["results"] = res

    x = np.zeros((B, N, NB), np.float32)
    for c in range(NCORES):
        xo = np.asarray(res.results[c]["xout"]).astype(np.float32)
        x[BL * c:BL * (c + 1)] = xo.reshape(NB, BL, N).transpose(1, 2, 0)
    return x



# revision 12
# speedup vs baseline: 1.3059x; 1.3059x over previous
"""Clifford ISTA kernel for 8 Trainium2 NeuronCores.

Strategy (data-parallel, zero cross-core communication, PE-roofline):
  - Shard batch B=64 across 8 cores (8 per core).
  - State and activations live in [A-row, (blade k, batch b)] layout so every
    main matmul uses the full 128x128 PE array: stationary operand = a 128x128
    chunk of A (or its negation), moving operand = x/err columns. The Clifford
    blade mixing (XOR permutation + Cayley sign) is folded into the moving
    operand's ACCESS PATTERN: for each j-blade of A, the (k,b) columns are
    read XOR-permuted via stepped/negative-stride AP dims, and the Cayley
    sign (constant on k-pairs, +-Walsh patterns) selects the negated
    stationary copy. Each (j, chunk) splits into <=2 sign-homogeneous
    strided pieces -> no prep matmuls, no transposes, no permute copies.
  - err = Ax - y and v = STEP*grad - x are produced directly in PSUM by
    folding the -y / -x adds into the accumulation group as identity-
    stationary f32/f32r matmuls (which double as the group's start=True).
  - Soft threshold on v = -u: c = min(max(v,-thr),thr); x_new = c - v;
    state kept negated (nx32, f32) with a bf16 shadow (xbf) for the PE.
    Host negates the output. Updates split across DVE and GPSIMD.
  - 50 iterations fully unrolled; iteration 0 specializes x=0 -> err=-y
    (preloaded) and skips FWD + x-fold.
"""

import os
import numpy as np
import ml_dtypes

# Problem constants (hardcoded per contest contract).
B, M, N, NB = 64, 256, 512, 8
BL = 8           # local batch per core
NCORES = 8
N_ITER = 50
STEP = 0.01
LAMBDAS = [0.0, 0.001, 0.001, 0.002]


def _cayley_sign():
    """cay[a, b] = C[a, b, a^b] for Cl(3,0): the canonical reordering sign."""
    cay = np.zeros((NB, NB), np.float32)
    for a in range(NB):
        for b in range(NB):
            cnt, aa = 0, a >> 1
            while aa:
                cnt += bin(aa & b).count("1")
                aa >>= 1
            cay[a, b] = -1.0 if (cnt & 1) else 1.0
    return cay


def _grades():
    return np.array([bin(i).count("1") for i in range(NB)], np.int32)


def _pieces():
    """Per j-blade: sign-homogeneous strided pieces over k-pair blocks.

    sig_j(k) = cay[k^j, j] is constant on pairs K = k>>1. A piece is a list
    of out-K blocks (ascending, strided) with constant sign whose source
    blocks K^(j>>1) are also strided. Returns {j: [(Ks, sign), ...]}.
    """
    cay = _cayley_sign()
    pieces = {}
    for j in range(NB):
        sigK = [cay[(2 * K) ^ j, j] for K in range(4)]

        def strided(seq):
            if len(seq) <= 1:
                return True
            d = seq[1] - seq[0]
            return all(seq[i + 1] - seq[i] == d for i in range(len(seq) - 1))

        Jhi = j >> 1
        allK = [0, 1, 2, 3]
        if len(set(sigK)) == 1 and strided([K ^ Jhi for K in allK]):
            pieces[j] = [(allK, sigK[0])]
            continue
        out = []
        for s in (1.0, -1.0):
            Ks = [K for K in allK if sigK[K] == s]
            if not Ks:
                continue
            if strided(Ks) and strided([K ^ Jhi for K in Ks]):
                out.append((Ks, s))
            else:
                h = len(Ks) // 2
                for part in (Ks[:h], Ks[h:]):
                    assert strided(part) and strided([K ^ Jhi for K in part])
                    out.append((part, s))
        pieces[j] = out
    return pieces


PIECES = _pieces()


def build_consts(A):
    """Host-side constant tensors shared by all cores."""
    A = np.asarray(A, np.float32)
    rev = ((-1.0) ** (_grades() * (_grades() - 1) // 2)).astype(np.float32)

    # atf2[p, ((j*4+q)*2+mc)*128 + m'] = A[128*mc+m', 128*q+p, j]
    atf2 = np.empty((128, 8192), np.float32)
    for j in range(NB):
        for q in range(4):
            for mc in range(2):
                blk = ((j * 4 + q) * 2 + mc) * 128
                atf2[:, blk:blk + 128] = A[128 * mc:128 * (mc + 1),
                                           128 * q:128 * (q + 1), j].T
    # abw2[p, ((j*2+r)*4+v)*128 + n'] = A[128*r+p, 128*v+n', j]*rev[j]*STEP
    abw2 = np.empty((128, 8192), np.float32)
    for j in range(NB):
        for r in range(2):
            for v in range(4):
                blk = ((j * 2 + r) * 4 + v) * 128
                abw2[:, blk:blk + 128] = (A[128 * r:128 * (r + 1),
                                            128 * v:128 * (v + 1), j]
                                          * (rev[j] * STEP))
    bf16 = ml_dtypes.bfloat16
    atf2 = atf2.astype(bf16)
    abw2 = abw2.astype(bf16)

    thr_row = np.repeat(np.array(LAMBDAS, np.float32)[_grades()], BL)  # [64]
    thrp = np.broadcast_to(thr_row, (128, 64)).copy()
    thrn = -thrp
    ident = np.eye(128, dtype=np.float32)
    return atf2, -atf2, abw2, -abw2, thrn, thrp, ident


def build_program(n_iter=N_ITER):
    """Build the per-core Bass/Tile program (identical on all cores)."""
    from contextlib import ExitStack
    import concourse.tile as tile
    from concourse import bacc, mybir

    f32 = mybir.dt.float32
    f32r = mybir.dt.float32r
    bf16 = mybir.dt.bfloat16
    ALU = mybir.AluOpType

    nc = bacc.Bacc(None, target_bir_lowering=False)

    atf2_d = nc.dram_tensor("atf2", [128, 8192], bf16, kind="ExternalInput")
    atf2n_d = nc.dram_tensor("atf2n", [128, 8192], bf16, kind="ExternalInput")
    abw2_d = nc.dram_tensor("abw2", [128, 8192], bf16, kind="ExternalInput")
    abw2n_d = nc.dram_tensor("abw2n", [128, 8192], bf16, kind="ExternalInput")
    ident_d = nc.dram_tensor("ident", [128, 128], f32r, kind="ExternalInput")
    nyt_d = nc.dram_tensor("nyt", [128, 128], f32r, kind="ExternalInput")
    err0_d = nc.dram_tensor("err0", [128, 128], bf16, kind="ExternalInput")
    thrn_d = nc.dram_tensor("thrn", [128, 64], f32, kind="ExternalInput")
    thrp_d = nc.dram_tensor("thrp", [128, 64], f32, kind="ExternalInput")
    xout_d = nc.dram_tensor("xout", [128, 256], f32, kind="ExternalOutput")

    with ExitStack() as ctx:
        tc = ctx.enter_context(tile.TileContext(nc))
        cpool = ctx.enter_context(tc.tile_pool(name="consts", bufs=1))
        wpool = ctx.enter_context(tc.tile_pool(name="work", bufs=2))
        ppool = ctx.enter_context(tc.tile_pool(name="ps", bufs=1, space="PSUM"))

        # ---- constant loads, ordered by first use (BWD of iteration 0) ----
        abw2_t = cpool.tile([128, 8192], bf16, name="abw2_t")
        abw2n_t = cpool.tile([128, 8192], bf16, name="abw2n_t")
        err0_t = cpool.tile([128, 128], bf16, name="err0_t")
        thrn_t = cpool.tile([128, 64], f32, name="thrn_t")
        thrp_t = cpool.tile([128, 64], f32, name="thrp_t")
        atf2_t = cpool.tile([128, 8192], bf16, name="atf2_t")
        atf2n_t = cpool.tile([128, 8192], bf16, name="atf2n_t")
        ident_t = cpool.tile([128, 128], f32r, name="ident_t")
        nyt_t = cpool.tile([128, 128], f32r, name="nyt_t")

        nc.sync.dma_start(err0_t[:], err0_d[:])
        for ch in range(4):
            sl = slice(2048 * ch, 2048 * (ch + 1))
            nc.sync.dma_start(abw2_t[:, sl], abw2_d[:, sl])
            nc.sync.dma_start(abw2n_t[:, sl], abw2n_d[:, sl])
        nc.sync.dma_start(thrn_t[:], thrn_d[:])
        nc.sync.dma_start(thrp_t[:], thrp_d[:])
        nc.sync.dma_start(ident_t[:], ident_d[:])
        nc.sync.dma_start(nyt_t[:], nyt_d[:])
        for ch in range(4):
            sl = slice(2048 * ch, 2048 * (ch + 1))
            nc.sync.dma_start(atf2_t[:, sl], atf2_d[:, sl])
            nc.sync.dma_start(atf2n_t[:, sl], atf2n_d[:, sl])

        # ---- state ----
        xbf = cpool.tile([128, 256], bf16, name="xbf")    # bf16 shadow of -nx32
        nx32 = cpool.tile([128, 256], f32r, name="nx32")  # f32 state, negated
        xfin = cpool.tile([128, 256], f32, name="xfin")   # final positive x

        def rhs_ap(t, cb, j, Ks):
            """Moving-operand AP: XOR-permuted (k,b) columns of t[:, cb:cb+64]
            for out-K blocks Ks (ascending), j's blade permutation."""
            v4 = t[:, cb:cb + 64].rearrange("p (K k0 b) -> p K k0 b", K=4, k0=2)
            src = [K ^ (j >> 1) for K in Ks]
            if len(src) == 1:
                ksl = slice(src[0], src[0] + 1)
            else:
                d = src[1] - src[0]
                stop = src[-1] + (1 if d > 0 else -1)
                ksl = slice(src[0], None if stop < 0 else stop, d)
            k0sl = slice(None, None, -1) if (j & 1) else slice(None)
            return v4[:, ksl, k0sl, :]

        def out_ap(pt, ob, Ks):
            u3 = pt[:, ob:ob + 64].rearrange("p (K x) -> p K x", K=4)
            d = (Ks[1] - Ks[0]) if len(Ks) > 1 else 1
            return u3[:, slice(Ks[0], Ks[-1] + 1, d), :]

        def emit_iteration(it):
            last_it = it == n_iter - 1
            if it > 0:
                # ---- FWD: psAx = Ax - y, accumulated in one PSUM group ----
                psAx = ppool.tile([128, 128], f32, name=f"psAx_{it}",
                                  tag="psAx", bufs=2)
                nc.tensor.matmul(psAx[:, :], lhsT=ident_t[:], rhs=nyt_t[:],
                                 start=True, stop=False)
                err_t = wpool.tile([128, 128], bf16, name=f"err_{it}",
                                   tag="err", bufs=2)
                nmm = sum(len(PIECES[j]) for j in range(NB)) * 4  # per mc
                for mc in range(2):
                    idx = 0
                    for q in range(4):
                        for j in range(NB):
                            blk = ((j * 4 + q) * 2 + mc) * 128
                            for Ks, s in PIECES[j]:
                                w = atf2_t if s > 0 else atf2n_t
                                idx += 1
                                nc.tensor.matmul(
                                    out_ap(psAx, 64 * mc, Ks),
                                    lhsT=w[:, blk:blk + 128],
                                    rhs=rhs_ap(xbf, 64 * q, j, Ks),
                                    start=False,
                                    stop=(mc == 1 and idx == nmm),
                                )
                    # err half ready as soon as its mc block completes
                    nc.scalar.copy(err_t[:, 64 * mc:64 * (mc + 1)],
                                   psAx[:, 64 * mc:64 * (mc + 1)])
                err_ap = err_t
            else:
                err_ap = err0_t

            # ---- BWD: psX = STEP*grad - x  (= -u), one PSUM group. The
            # x-fold sits between the r=0 and r=1 piece blocks: late enough
            # that the previous iteration's deferred nx32 writes are done,
            # early enough not to delay the per-v UPDATE pipeline. ----
            psX = ppool.tile([128, 256], f32, name=f"psX_{it}",
                             tag="psX", bufs=2)
            nmm = sum(len(PIECES[j]) for j in range(NB)) * 8
            idx = 0
            for r in range(2):
                if r == 1 and it > 0:
                    nc.tensor.matmul(psX[:, :], lhsT=ident_t[:], rhs=nx32[:],
                                     start=False, stop=False)
                for v in range(4):
                    for j in range(NB):
                        blk = ((j * 2 + r) * 4 + v) * 128
                        for Ks, s in PIECES[j]:
                            w = abw2_t if s > 0 else abw2n_t
                            idx += 1
                            nc.tensor.matmul(
                                out_ap(psX, 64 * v, Ks),
                                lhsT=w[:, blk:blk + 128],
                                rhs=rhs_ap(err_ap, 64 * r, j, Ks),
                                start=(idx == 1),
                                stop=(idx == nmm),
                            )

            # ---- UPDATE per v-chunk (DVE): c = clamp(v,-thr,thr);
            # x_new = c - v. The f32 state write (nx32 = v - c) feeds only
            # the NEXT iteration's x-fold, so all four are deferred past the
            # xbf writes that gate the next FWD. ----
            ctiles = []
            for v in range(4):
                sl = slice(64 * v, 64 * (v + 1))
                c = wpool.tile([128, 64], f32, name=f"c{v}_{it}",
                               tag=f"c{v}", bufs=2)
                ctiles.append(c)
                nc.vector.tensor_max(c[:], psX[:, sl], thrn_t[:])
                nc.vector.tensor_tensor(c[:], c[:], thrp_t[:], ALU.min)
                if last_it:
                    # final iteration: positive x, straight to the output tile
                    nc.vector.tensor_sub(xfin[:, sl], c[:], psX[:, sl])
                else:
                    nc.vector.tensor_sub(xbf[:, sl], c[:], psX[:, sl])
            if not last_it:
                for v in range(4):
                    sl = slice(64 * v, 64 * (v + 1))
                    nc.vector.tensor_sub(nx32[:, sl], psX[:, sl], ctiles[v][:])

        for it in range(n_iter):
            emit_iteration(it)

        nc.sync.dma_start(xout_d[:], xfin[:])

    nc.compile()
    return nc


_program_cache = {}


def _get_program(n_iter):
    if n_iter not in _program_cache:
        _program_cache[n_iter] = build_program(n_iter)
    return _program_cache[n_iter]


LAST_INFO = {}


def kernel(y, A, _trace=False, _n_iter=None):
    y = np.asarray(y, np.float32)
    A = np.asarray(A, np.float32)
    n_iter = N_ITER if _n_iter is None else _n_iter

    from concourse.bass_utils import run_bass_kernel_spmd

    nc = _get_program(n_iter)
    atf2, atf2n, abw2, abw2n, thrn, thrp, ident = build_consts(A)

    in_maps = []
    for c in range(NCORES):
        ysl = y[BL * c:BL * (c + 1)]                      # [8, 256, 8] (b,m,k)
        # nyt[p, mc*64 + k*8 + b] = -y[b, 128*mc+p, k]
        nyt = np.ascontiguousarray(
            (-ysl).transpose(1, 2, 0).reshape(2, 128, 64).transpose(1, 0, 2)
            .reshape(128, 128))
        in_maps.append({
            "atf2": atf2, "atf2n": atf2n, "abw2": abw2, "abw2n": abw2n,
            "ident": ident, "nyt": nyt,
            "err0": nyt.astype(ml_dtypes.bfloat16),
            "thrn": thrn, "thrp": thrp,
        })

    try:
        res = run_bass_kernel_spmd(
            nc, in_maps, core_ids=list(range(NCORES)), trace=_trace,
        )
    except ModuleNotFoundError:
        res = run_bass_kernel_spmd(
            nc, in_maps, core_ids=list(range(NCORES)), trace=False,
        )
    LAST_INFO["exec_time_ns"] = res.exec_time_ns
    LAST_INFO["results"] = res

    x = np.zeros((B, N, NB), np.float32)
    for c in range(NCORES):
        xo = np.asarray(res.results[c]["xout"]).astype(np.float32)
        # x[b, 128*v+p, k] = xo[p, v*64 + k*8 + b]
        xr = xo.reshape(128, 4, 8, 8)                     # [p, v, k, b]
        x[BL * c:BL * (c + 1)] = xr.transpose(3, 1, 0, 2).reshape(8, 512, 8)
    return x


# revision 42
# speedup vs baseline: 2.2089x; 1.6915x over previous
"""Clifford ISTA kernel for 8 Trainium2 NeuronCores.

Strategy (data-parallel, zero cross-core communication, PE-roofline):
  - Shard batch B=64 across 8 cores (8 per core).
  - State and activations live in [A-row, (blade k, batch b)] layout so every
    main matmul uses the full 128x128 PE array: stationary operand = a 128x128
    chunk of A (or its negation), moving operand = x/err columns. The Clifford
    blade mixing (XOR permutation + Cayley sign) is folded into the moving
    operand's ACCESS PATTERN: for each j-blade of A, the (k,b) columns are
    read XOR-permuted via stepped/negative-stride AP dims, and the Cayley
    sign (constant on k-pairs, +-Walsh patterns) selects the negated
    stationary copy. Each (j, chunk) splits into <=2 sign-homogeneous
    strided pieces -> no prep matmuls, no transposes, no permute copies.
  - err = Ax - y and v = STEP*grad - x are produced directly in PSUM by
    folding the -y / -x adds into the accumulation group as identity-
    stationary f32/f32r matmuls (which double as the group's start=True).
  - Soft threshold on v = -u: c = min(max(v,-thr),thr); x_new = c - v;
    state kept negated (nx32, f32) with a bf16 shadow (xbf) for the PE.
    Host negates the output. Updates split across DVE and GPSIMD.
  - 50 iterations fully unrolled; iteration 0 specializes x=0 -> err=-y
    (preloaded) and skips FWD + x-fold.
"""

import os
import numpy as np
import ml_dtypes

# Problem constants (hardcoded per contest contract).
B, M, N, NB = 64, 256, 512, 8
BL = 8           # local batch per core
NCORES = 8
N_ITER = 50
STEP = 0.01
LAMBDAS = [0.0, 0.001, 0.001, 0.002]


def _cayley_sign():
    """cay[a, b] = C[a, b, a^b] for Cl(3,0): the canonical reordering sign."""
    cay = np.zeros((NB, NB), np.float32)
    for a in range(NB):
        for b in range(NB):
            cnt, aa = 0, a >> 1
            while aa:
                cnt += bin(aa & b).count("1")
                aa >>= 1
            cay[a, b] = -1.0 if (cnt & 1) else 1.0
    return cay


def _grades():
    return np.array([bin(i).count("1") for i in range(NB)], np.int32)


def _pieces():
    """Per j-blade: sign-homogeneous strided pieces over k-pair blocks.

    sig_j(k) = cay[k^j, j] is constant on pairs K = k>>1. A piece is a list
    of out-K blocks (ascending, strided) with constant sign whose source
    blocks K^(j>>1) are also strided. Returns {j: [(Ks, sign), ...]}.
    """
    cay = _cayley_sign()
    pieces = {}
    for j in range(NB):
        sigK = [cay[(2 * K) ^ j, j] for K in range(4)]

        def strided(seq):
            if len(seq) <= 1:
                return True
            d = seq[1] - seq[0]
            return all(seq[i + 1] - seq[i] == d for i in range(len(seq) - 1))

        Jhi = j >> 1
        allK = [0, 1, 2, 3]
        if len(set(sigK)) == 1 and strided([K ^ Jhi for K in allK]):
            pieces[j] = [(allK, sigK[0])]
            continue
        out = []
        for s in (1.0, -1.0):
            Ks = [K for K in allK if sigK[K] == s]
            if not Ks:
                continue
            if strided(Ks) and strided([K ^ Jhi for K in Ks]):
                out.append((Ks, s))
            else:
                h = len(Ks) // 2
                for part in (Ks[:h], Ks[h:]):
                    assert strided(part) and strided([K ^ Jhi for K in part])
                    out.append((part, s))
        pieces[j] = out
    return pieces


PIECES = _pieces()


def build_consts(A):
    """Host-side constant tensors shared by all cores."""
    A = np.asarray(A, np.float32)
    rev = ((-1.0) ** (_grades() * (_grades() - 1) // 2)).astype(np.float32)

    # atf2[p, ((q*2+mc)*8+j)*128 + m'] = A[128*mc+m', 128*q+p, j]
    # (q-major to match the FWD's q-block consumption order, so the first
    # chunk DMAs unblock the first FWD blocks)
    atf2 = np.empty((128, 8192), np.float32)
    for j in range(NB):
        for q in range(4):
            for mc in range(2):
                blk = ((q * 2 + mc) * 8 + j) * 128
                atf2[:, blk:blk + 128] = A[128 * mc:128 * (mc + 1),
                                           128 * q:128 * (q + 1), j].T
    # abw2[p, ((v*2+r)*8+j)*128 + n'] = A[128*r+p, 128*v+n', j]*rev[j]*STEP
    # (v-major to match the BWD's (v, r) block consumption order)
    abw2 = np.empty((128, 8192), np.float32)
    for j in range(NB):
        for r in range(2):
            for v in range(4):
                blk = ((v * 2 + r) * 8 + j) * 128
                abw2[:, blk:blk + 128] = (A[128 * r:128 * (r + 1),
                                            128 * v:128 * (v + 1), j]
                                          * (rev[j] * STEP))
    bf16 = ml_dtypes.bfloat16
    atf2 = atf2.astype(bf16)
    abw2 = abw2.astype(bf16)
    ident = np.eye(128, dtype=np.float32)
    return atf2, abw2, ident


def build_program(n_iter=N_ITER):
    """Build the per-core Bass/Tile program (identical on all cores)."""
    from contextlib import ExitStack
    import concourse.tile as tile
    from concourse import bacc, mybir

    f32 = mybir.dt.float32
    f32r = mybir.dt.float32r
    bf16 = mybir.dt.bfloat16
    ALU = mybir.AluOpType

    nc = bacc.Bacc(None, target_bir_lowering=False)

    atf2_d = nc.dram_tensor("atf2", [128, 8192], bf16, kind="ExternalInput")
    abw2_d = nc.dram_tensor("abw2", [128, 8192], bf16, kind="ExternalInput")
    ident_d = nc.dram_tensor("ident", [128, 128], f32, kind="ExternalInput")
    nyt_d = nc.dram_tensor("nyt", [128, 128], f32, kind="ExternalInput")
    err0_d = nc.dram_tensor("err0", [128, 128], bf16, kind="ExternalInput")
    xout_d = nc.dram_tensor("xout", [128, 256], f32, kind="ExternalOutput")

    with ExitStack() as ctx:
        tc = ctx.enter_context(tile.TileContext(nc))
        cpool = ctx.enter_context(tc.tile_pool(name="consts", bufs=1))
        wpool = ctx.enter_context(tc.tile_pool(name="work", bufs=2))
        ppool = ctx.enter_context(tc.tile_pool(name="ps", bufs=1, space="PSUM"))

        # ---- constant loads. All DMA transfers serialize on the global DMA
        # device in the cost model, so order strictly by first use; the
        # negated weight copies are derived on-device by DVE (idle during
        # load) instead of doubling the HBM traffic. ----
        abw2_t = cpool.tile([128, 8192], bf16, name="abw2_t")
        abw2n_t = cpool.tile([128, 8192], bf16, name="abw2n_t")
        err0_t = cpool.tile([128, 128], bf16, name="err0_t")
        atf2_t = cpool.tile([128, 8192], bf16, name="atf2_t")
        atf2n_t = cpool.tile([128, 8192], bf16, name="atf2n_t")
        ident_t = cpool.tile([128, 128], f32, name="ident_t")
        nyt_t = cpool.tile([128, 128], f32, name="nyt_t")

        nc.sync.dma_start(err0_t[:], err0_d[:])
        for ch in range(4):
            sl = slice(2048 * ch, 2048 * (ch + 1))
            nc.sync.dma_start(abw2_t[:, sl], abw2_d[:, sl])
            nc.vector.tensor_scalar_mul(abw2n_t[:, sl], abw2_t[:, sl], -1.0)
        nc.sync.dma_start(ident_t[:], ident_d[:])
        nc.sync.dma_start(nyt_t[:], nyt_d[:])
        for ch in range(4):
            sl = slice(2048 * ch, 2048 * (ch + 1))
            nc.sync.dma_start(atf2_t[:, sl], atf2_d[:, sl])
            nc.vector.tensor_scalar_mul(atf2n_t[:, sl], atf2_t[:, sl], -1.0)

        # ---- state ----
        xbf = cpool.tile([128, 256], bf16, name="xbf")    # bf16 shadow of -nx32
        nx32 = cpool.tile([128, 256], f32, name="nx32")   # f32 state, negated
        xfin = cpool.tile([128, 256], f32, name="xfin")   # final positive x
        # persistent clamp tiles; k=0 columns (thr=0 -> clamp=0) are zeroed
        # once so the per-iteration update touches them never and the xbf
        # write is a single full-width op per half
        ctile = [cpool.tile([128, 128], f32, name=f"ctile{h}") for h in range(2)]
        for h in range(2):
            cg = ctile[h][:].rearrange("p (g x) -> p g x", g=2)
            nc.vector.memset(cg[:, :, 0:8], 0.0)

        def rhs_ap(t, cb, j, Ks):
            """Moving-operand AP: XOR-permuted (k,b) columns of t[:, cb:cb+64]
            for out-K blocks Ks (ascending), j's blade permutation."""
            v4 = t[:, cb:cb + 64].rearrange("p (K k0 b) -> p K k0 b", K=4, k0=2)
            src = [K ^ (j >> 1) for K in Ks]
            if len(src) == 1:
                ksl = slice(src[0], src[0] + 1)
            else:
                d = src[1] - src[0]
                stop = src[-1] + (1 if d > 0 else -1)
                ksl = slice(src[0], None if stop < 0 else stop, d)
            k0sl = slice(None, None, -1) if (j & 1) else slice(None)
            return v4[:, ksl, k0sl, :]

        def out_ap(pt, ob, Ks):
            u3 = pt[:, ob:ob + 64].rearrange("p (K x) -> p K x", K=4)
            d = (Ks[1] - Ks[0]) if len(Ks) > 1 else 1
            return u3[:, slice(Ks[0], Ks[-1] + 1, d), :]

        def _lab(inst, label):
            LABELS[inst.ins.name] = label
            return inst

        T1, T2 = LAMBDAS[1], LAMBDAS[3]  # grade-1/2 and grade-3 thresholds

        def kgroups(t, h):
            """[p, g, x] view of half h (two 64-col (k,b) groups) of t."""
            return t[:, 128 * h:128 * (h + 1)].rearrange(
                "p (g x) -> p g x", g=2)

        def emit_iteration(it):
            last_it = it == n_iter - 1
            if it > 0:
                # ---- FWD: psAx[mc] = Ax - y, one PSUM group per mc-half so
                # each err copy fires as soon as its own half stops. The -y
                # add is folded in as an identity matmul doubling as the
                # group's start. q-major/mc-inner order consumes xbf halves
                # in the order the update pipeline produces them. ----
                psAx = [ppool.tile([128, 64], f32, name=f"psAx{mc}_{it}",
                                   tag=f"psAx{mc}", bufs=2) for mc in range(2)]
                for mc in range(2):
                    _lab(nc.tensor.matmul(psAx[mc][:, :], lhsT=ident_t[:],
                                     rhs=nyt_t[:, 64 * mc:64 * (mc + 1)],
                                     start=True, stop=False), f"yfold{mc}_{it}")
                err_t = [wpool.tile([128, 64], bf16, name=f"err{mc}_{it}",
                                    tag=f"err{mc}", bufs=2) for mc in range(2)]
                nmm = sum(len(PIECES[j]) for j in range(NB)) * 4  # per mc
                for mc in range(2):
                    idx = 0
                    for q in range(4):
                        for j in range(NB):
                            blk = ((q * 2 + mc) * 8 + j) * 128
                            for Ks, s in PIECES[j]:
                                w = atf2_t if s > 0 else atf2n_t
                                idx += 1
                                _lab(nc.tensor.matmul(
                                    out_ap(psAx[mc], 0, Ks),
                                    lhsT=w[:, blk:blk + 128],
                                    rhs=rhs_ap(xbf, 64 * q, j, Ks),
                                    start=False,
                                    stop=(idx == nmm),
                                ), f"fwd_q{q}mc{mc}j{j}_{it}")
                    # err half ready at FWD midpoint (mc-major order)
                    _lab(nc.scalar.copy(err_t[mc][:, :], psAx[mc][:, :]),
                         f"errcopy{mc}_{it}")
                err_ap = err_t
            else:
                err_ap = [err0_t[:, 0:64], err0_t[:, 64:128]]

            # ---- BWD: psX[h] = STEP*grad - x (= -u), one PSUM group per
            # v-pair half so the h=0 update chain starts at the BWD midpoint.
            # The x-fold (identity matmul adding -x) doubles as each group's
            # start; err1 isn't needed until three blocks into h=0. ----
            psX = [ppool.tile([128, 128], f32, name=f"psX{h}_{it}",
                              tag=f"psX{h}", bufs=2) for h in range(2)]
            nmm = sum(len(PIECES[j]) for j in range(NB)) * 2  # per (v, r)
            for h in range(2):
                if it > 0:
                    _lab(nc.tensor.matmul(psX[h][:, :], lhsT=ident_t[:],
                                     rhs=nx32[:, 128 * h:128 * (h + 1)],
                                     start=True, stop=False), f"xfold{h}_{it}")
                idx = 0
                for v, r in ((0, 0), (1, 0), (0, 1), (1, 1)):
                    v += 2 * h
                    idx += 1
                    for j in range(NB):
                        blk = ((v * 2 + r) * 8 + j) * 128
                        for Ks, s in PIECES[j]:
                            w = abw2_t if s > 0 else abw2n_t
                            _lab(nc.tensor.matmul(
                                out_ap(psX[h], 64 * (v - 2 * h), Ks),
                                lhsT=w[:, blk:blk + 128],
                                rhs=rhs_ap(err_ap[r], 0, j, Ks),
                                start=(it == 0 and idx == 1
                                       and j == 0 and Ks[0] == 0),
                                stop=(idx == 4 and j == NB - 1),
                            ), f"bwd_h{h}v{v}r{r}j{j}_{it}")

            # ---- UPDATE (DVE) per half: x_new = c - v with c = clamp(v)
            # fused into one dual-op tensor_scalar per threshold group
            # (k=0 has thr=0 so x_new = -v directly). The f32 state write
            # (nx32 = v - c, and +v for k=0) only feeds the NEXT iteration's
            # x-fold, so both halves are deferred past the xbf writes that
            # gate the next FWD. ----
            ctiles = []
            for h in range(2):
                pg = psX[h][:].rearrange("p (g x) -> p g x", g=2)
                cg = ctile[h][:].rearrange("p (g x) -> p g x", g=2)
                ctiles.append((pg, cg))
                _lab(nc.vector.tensor_scalar(cg[:, :, 8:56], pg[:, :, 8:56],
                                        -T1, T1, ALU.max, ALU.min), f"ts1_h{h}_{it}")
                _lab(nc.vector.tensor_scalar(cg[:, :, 56:64], pg[:, :, 56:64],
                                        -T2, T2, ALU.max, ALU.min), f"ts2_h{h}_{it}")
                xt = xfin if last_it else xbf
                _lab(nc.vector.tensor_sub(xt[:, 128 * h:128 * (h + 1)],
                                          ctile[h][:], psX[h][:]),
                     f"xbfsub_h{h}_{it}")
            if not last_it:
                for h in range(2):
                    _lab(nc.vector.tensor_sub(nx32[:, 128 * h:128 * (h + 1)],
                                              psX[h][:], ctile[h][:]),
                         f"nxsub_h{h}_{it}")

        for it in range(n_iter):
            emit_iteration(it)

        # per-half output DMA so the first half ships while the second
        # half's update is still finishing
        nc.sync.dma_start(xout_d[:, 0:128], xfin[:, 0:128])
        nc.sync.dma_start(xout_d[:, 128:256], xfin[:, 128:256])

    nc.compile()
    return nc


_program_cache = {}


def _get_program(n_iter):
    if n_iter not in _program_cache:
        _program_cache[n_iter] = build_program(n_iter)
    return _program_cache[n_iter]


LAST_INFO = {}
LABELS = {}


def kernel(y, A, _trace=False, _n_iter=None):
    y = np.asarray(y, np.float32)
    A = np.asarray(A, np.float32)
    n_iter = N_ITER if _n_iter is None else _n_iter

    from concourse.bass_utils import run_bass_kernel_spmd

    nc = _get_program(n_iter)
    atf2, abw2, ident = build_consts(A)

    in_maps = []
    for c in range(NCORES):
        ysl = y[BL * c:BL * (c + 1)]                      # [8, 256, 8] (b,m,k)
        # nyt[p, mc*64 + k*8 + b] = -y[b, 128*mc+p, k]
        nyt = np.ascontiguousarray(
            (-ysl).transpose(1, 2, 0).reshape(2, 128, 64).transpose(1, 0, 2)
            .reshape(128, 128))
        in_maps.append({
            "atf2": atf2, "abw2": abw2,
            "ident": ident, "nyt": nyt,
            "err0": nyt.astype(ml_dtypes.bfloat16),
        })

    try:
        res = run_bass_kernel_spmd(
            nc, in_maps, core_ids=list(range(NCORES)), trace=_trace,
        )
    except ModuleNotFoundError:
        res = run_bass_kernel_spmd(
            nc, in_maps, core_ids=list(range(NCORES)), trace=False,
        )
    LAST_INFO["exec_time_ns"] = res.exec_time_ns
    LAST_INFO["results"] = res

    x = np.zeros((B, N, NB), np.float32)
    for c in range(NCORES):
        xo = np.asarray(res.results[c]["xout"]).astype(np.float32)
        # x[b, 128*v+p, k] = xo[p, v*64 + k*8 + b]
        xr = xo.reshape(128, 4, 8, 8)                     # [p, v, k, b]
        x[BL * c:BL * (c + 1)] = xr.transpose(3, 1, 0, 2).reshape(8, 512, 8)
    return x
